# revision 2
# baseline (speedup 1.0000x reference)
"""Grouped-scale dequant GEMM (AxCoreLinearFP16) on 8 Trainium2 NeuronCores.

y[b,s,o] = sum_i x[b,s,i] * (weight[o,i] * scales[o, i//128])

Strategy: data-parallel over the flattened (b*s) rows — each core gets a
[1024, 4096] x-shard and the full weight/scales (no collectives). Per core:
  - x^T resident in SBUF via four 2 MiB DMA transposes (contraction dim on
    partitions: xT[p, ko, m] = x[m, ko*128+p]); 4 chunks so the first
    matmuls can start after ~8 µs instead of ~35.
  - w^T o-panels (512 wide) DMA-transposed in two 2 MiB chunks each.
  - dequant: the host pre-transposes scales to panel-major [KO, OC] slices
    and replicates them across 128 partitions ("sb" input, 32 MiB). Per
    panel ONE plain 4 MiB DMA loads the broadcast scales, and two all-fp16
    DVE tensor_muls (2x-rate: 16-bit, step-1 operands) dequantize the w^T
    panel in place. This keeps the PE stream free of the rank-1 broadcast
    matmuls the previous version used, and keeps DVE at 2x (the old
    fp32-PSUM operand forced 1x and made dequant the critical path).
  - PE matmul accumulates over the 32 k-chunks into PSUM [128, 512] fp32;
    8-bank psum pool; PSUM evicted with casting ACT copies into a 2-panel
    staging buffer, stored to DRAM as [128, 2, 512] tiles (2 KiB contiguous
    per output row). Direct per-panel 1 KiB-row stores measurably push the
    chip into a power state where the PE drops 2.4 -> 2.0 GHz (matmuls
    455 -> 259... sorry, 216 -> 259 ns); the staged layout avoids that.

Workarounds for this environment's toolchain:
  - walrus here accepts only ONE sync-wait per instruction: extra waits are
    peeled onto same-engine NoOps (_split_multiwait_insts)

Self-contained: hardcodes shapes from the problem spec.
"""

import sys

for _p in ("/opt/trn_rl_repo",):
    if _p not in sys.path:
        sys.path.insert(0, _p)

from contextlib import ExitStack

import numpy as np

import concourse.bass as bass
import concourse.mybir as mybir
import concourse.tile as tile
import bass_rust


FP16 = mybir.dt.float16
FP32 = mybir.dt.float32

P = 128
NCORES = 8
B, S, IN, OUT = 4, 2048, 4096, 4096
GROUP = 128
M = B * S // NCORES          # 1024 rows of x per core
KO = IN // P                 # 32 k-chunks == quant groups
OC = 512                     # o-chunk (matmul free dim)
NOC = OUT // OC              # 8
MT = M // P                  # 8 m-tiles
SPAN = 2                     # panels staged per store

_RUNNER = None


def _split_multiwait_insts(nc):
    """This env's walrus CoreV3 codegen accepts only one sync-wait per
    instruction; Tile's tail drain can carry one per DMAHW sem lane.
    Peel extra waits onto same-engine NoOps inserted just before."""
    ctr = 0
    for f in nc.m.functions:
        for bb in f.blocks:
            new = []
            for inst in bb.instructions:
                si = inst.sync_info
                if si is not None and si.on_wait and len(si.on_wait) > 1:
                    waits = list(si.on_wait)
                    for w in waits[:-1]:
                        ctr += 1
                        new.append(bass_rust.InstNoOp(
                            name=f"I-waitsplit-{ctr}",
                            engine=inst.engine,
                            sync_info=bass_rust.SyncInfo(on_wait=[w], on_update=[]),
                        ))
                    inst.sync_info = bass_rust.SyncInfo(
                        on_wait=[waits[-1]], on_update=list(si.on_update or [])
                    )
                new.append(inst)
            bb.instructions = new
    return ctr


def _build(scb_bufs=1, wdeq_bufs=2, psum_bufs=8, xchunks=4, mul_split=2):
    nc = bass.Bass()
    x = nc.declare_dram_parameter("x", [M, IN], FP16, isOutput=False)
    w = nc.declare_dram_parameter("w", [OUT, IN], FP16, isOutput=False)
    sb = nc.declare_dram_parameter("sb", [P, NOC * KO * OC], FP16, isOutput=False)
    y = nc.declare_dram_parameter("y", [M, OUT], FP16, isOutput=True)

    with tile.TileContext(nc) as tc, ExitStack() as ctx:
        xTp = ctx.enter_context(tc.tile_pool(name="xTp", bufs=1))
        wraw = ctx.enter_context(tc.tile_pool(name="wraw", bufs=wdeq_bufs))
        scbp = ctx.enter_context(tc.tile_pool(name="scbp", bufs=scb_bufs))
        ystg = ctx.enter_context(tc.tile_pool(name="ystg", bufs=2))
        psum = ctx.enter_context(tc.tile_pool(name="psum", bufs=psum_bufs,
                                              space="PSUM"))

        xT = xTp.tile([P, KO, M], FP16)

        def emit_xchunk(i):
            xc = KO // xchunks
            nc.sync.dma_start_transpose(out=xT[:, i * xc:(i + 1) * xc, :],
                                        in_=x[:, i * xc * P:(i + 1) * xc * P])

        def emit_prefetch(oc):
            """Transpose w panel, load its broadcast scales, dequantize."""
            osl = slice(oc * OC, (oc + 1) * OC)
            wr3 = wraw.tile([P, KO, OC], FP16, tag="wraw", name="wr3")
            scb = scbp.tile([P, KO, OC], FP16, tag="scb", name="scb")
            nc.scalar.dma_start(out=scb[:],
                                in_=sb[:, oc * KO * OC:(oc + 1) * KO * OC])
            KH = KO // mul_split
            for g in range(mul_split):
                kg = slice(g * KH, (g + 1) * KH)
                nc.sync.dma_start_transpose(
                    out=wr3[:, kg, :], in_=w[osl, g * KH * P:(g + 1) * KH * P])
                nc.vector.tensor_mul(wr3[:, kg, :], wr3[:, kg, :], scb[:, kg, :])
            return wr3

        stage = {}

        def emit_compute(oc, wr3):
            grp, ph = oc // SPAN, oc % SPAN
            if ph == 0:
                stage[grp] = ystg.tile([P, MT, SPAN, OC], FP16, name="yst")
            yst = stage[grp]
            for m in range(MT):
                pt = psum.tile([P, OC], FP32, name="pt")
                for ko in range(KO):
                    nc.tensor.matmul(
                        pt[:],
                        xT[:, ko, m * P:(m + 1) * P],
                        wr3[:, ko, :],
                        start=(ko == 0),
                        stop=(ko == KO - 1),
                    )
                nc.scalar.copy(out=yst[:, m, ph, :], in_=pt[:])
            if ph == SPAN - 1:
                for m in range(MT):
                    nc.scalar.dma_start(
                        out=y[m * P:(m + 1) * P,
                              grp * SPAN * OC:(grp + 1) * SPAN * OC],
                        in_=yst[:, m, :, :])

        emit_xchunk(0)
        cur = emit_prefetch(0)
        for i in range(1, xchunks):
            emit_xchunk(i)
        for oc in range(NOC):
            nxt = emit_prefetch(oc + 1) if oc + 1 < NOC else None
            emit_compute(oc, cur)
            cur = nxt

    _split_multiwait_insts(nc)
    return nc


def make_sb(scales):
    """[OUT, KO] fp16 scales -> [128, NOC*KO*OC] panel-major broadcast."""
    pan = np.ascontiguousarray(
        np.asarray(scales, dtype=np.float16).T.reshape(KO, NOC, OC)
        .transpose(1, 0, 2))
    return np.ascontiguousarray(
        np.broadcast_to(pan.reshape(1, -1), (P, NOC * KO * OC)))


def make_in_maps(x, weight, scales):
    """Full inputs -> per-core input maps for the bass kernel."""
    xf = np.ascontiguousarray(np.asarray(x, dtype=np.float16).reshape(B * S, IN))
    w = np.ascontiguousarray(np.asarray(weight, dtype=np.float16))
    sbv = make_sb(scales)
    return [
        {"x": xf[c * M:(c + 1) * M], "w": w, "sb": sbv} for c in range(NCORES)
    ]


def _get_runner():
    """Compile once; return a reusable callable mapping per-core input maps
    to per-core output maps (modeled on bass2jax.run_bass_via_pjrt)."""
    global _RUNNER
    if _RUNNER is not None:
        return _RUNNER

    import jax
    from jax.experimental.shard_map import shard_map
    from jax.sharding import Mesh, PartitionSpec
    from concourse import bass2jax

    nc = _build()
    bass2jax.install_neuronx_cc_hook()

    partition_name = nc.partition_id_tensor.name if nc.partition_id_tensor else None
    in_names, out_names, out_avals, zero_shapes = [], [], [], []
    for alloc in nc.m.functions[0].allocations:
        if not isinstance(alloc, mybir.MemoryLocationSet):
            continue
        name = alloc.memorylocations[0].name
        if alloc.kind == "ExternalInput":
            if name != partition_name:
                in_names.append(name)
        elif alloc.kind == "ExternalOutput":
            shape = tuple(alloc.tensor_shape)
            dtype = mybir.dt.np(alloc.dtype)
            out_names.append(name)
            out_avals.append(jax.core.ShapedArray(shape, dtype))
            zero_shapes.append((shape, dtype))
    n_params = len(in_names)
    n_outs = len(out_names)
    all_names = in_names + out_names
    if partition_name is not None:
        all_names = all_names + [partition_name]
    donate = tuple(range(n_params, n_params + n_outs))

    def _make_body(reps):
        def _body(*args):
            ins = list(args[:n_params])
            outs = list(args[n_params:n_params + n_outs])
            for _ in range(reps):
                operands = ins + outs
                if partition_name is not None:
                    operands.append(bass2jax.partition_id_tensor())
                outs = list(bass2jax._bass_exec_p.bind(
                    *operands,
                    out_avals=tuple(out_avals),
                    in_names=tuple(all_names),
                    out_names=tuple(out_names),
                    lowering_input_output_aliases=(),
                    sim_require_finite=True,
                    sim_require_nnan=True,
                    nc=nc,
                ))
            return tuple(outs)
        return _body

    devices = jax.devices()[:NCORES]
    mesh = Mesh(np.asarray(devices), ("core",))

    def _make_exec(reps):
        return jax.jit(
            shard_map(
                _make_body(reps),
                mesh=mesh,
                in_specs=(PartitionSpec("core"),) * (n_params + n_outs),
                out_specs=(PartitionSpec("core"),) * n_outs,
                check_rep=False,
            ),
            donate_argnums=donate,
            keep_unused=True,
        )

    sharded = _make_exec(1)
    _exec_cache = {1: sharded}
    from jax.sharding import NamedSharding
    shard = NamedSharding(mesh, PartitionSpec("core"))

    class Runner:
        def __init__(self):
            self.in_names = in_names
            self.out_names = out_names

        def put_inputs(self, in_maps):
            """Concat per-core inputs and place them on the mesh."""
            import jax as _jax
            concat_in = [
                np.concatenate([np.asarray(m[name]) for m in in_maps], axis=0)
                for name in in_names
            ]
            return [_jax.device_put(a, shard) for a in concat_in]

        def fresh_outs(self):
            import jax as _jax
            return [
                _jax.device_put(np.zeros((NCORES * sh[0], *sh[1:]), dt), shard)
                for sh, dt in zero_shapes
            ]

        def exec_dev(self, dev_in, dev_outs, reps=1):
            """Device step(s). dev_outs is donated; returns new out arrays
            (same shape/sharding — reusable as the next call's dev_outs,
            since the kernel overwrites every output element). reps>1
            chains that many NEFF executions inside one dispatch."""
            if reps not in _exec_cache:
                _exec_cache[reps] = _make_exec(reps)
            return _exec_cache[reps](*dev_in, *dev_outs)

        def run(self, in_maps):
            dev_in = self.put_inputs(in_maps)
            out_arrs = self.exec_dev(dev_in, self.fresh_outs())
            return [
                {
                    name: np.asarray(out_arrs[i]).reshape(
                        NCORES, *out_avals[i].shape)[c]
                    for i, name in enumerate(out_names)
                }
                for c in range(NCORES)
            ]

    _RUNNER = Runner()
    return _RUNNER


def kernel(x, weight, scales):
    runner = _get_runner()
    in_maps = make_in_maps(x, weight, scales)
    outs = runner.run(in_maps)
    yf = np.concatenate([outs[c]["y"] for c in range(NCORES)], axis=0)
    return yf.reshape(B, S, OUT).astype(np.float16)


# revision 9
# speedup vs baseline: 1.8734x; 1.8734x over previous
"""Grouped-scale dequant GEMM (AxCoreLinearFP16) on 8 Trainium2 NeuronCores.

y[b,s,o] = sum_i x[b,s,i] * (weight[o,i] * scales[o, i//128])

Strategy: data-parallel over the flattened (b*s) rows — each core gets a
[1024, 4096] x-shard and the full weight/scales (no collectives). Per core:
  - x^T resident in SBUF via four 2 MiB DMA transposes (contraction dim on
    partitions: xT[p, ko, m] = x[m, ko*128+p]); 4 chunks so the first
    matmuls can start after ~8 µs instead of ~35.
  - w^T o-panels (512 wide) DMA-transposed in two 2 MiB chunks each.
  - dequant: the host pre-transposes scales to panel-major [KO, OC] slices
    and replicates them across 128 partitions ("sb" input, 32 MiB). Per
    panel ONE plain 4 MiB DMA loads the broadcast scales, and two all-fp16
    DVE tensor_muls (2x-rate: 16-bit, step-1 operands) dequantize the w^T
    panel in place. This keeps the PE stream free of the rank-1 broadcast
    matmuls the previous version used, and keeps DVE at 2x (the old
    fp32-PSUM operand forced 1x and made dequant the critical path).
  - PE matmul accumulates over the 32 k-chunks into PSUM [128, 512] fp32;
    8-bank psum pool; PSUM evicted with casting ACT copies into a 2-panel
    staging buffer, stored to DRAM as [128, 2, 512] tiles (2 KiB contiguous
    per output row). Direct per-panel 1 KiB-row stores measurably push the
    chip into a power state where the PE drops 2.4 -> 2.0 GHz (matmuls
    455 -> 259... sorry, 216 -> 259 ns); the staged layout avoids that.

Workarounds for this environment's toolchain:
  - walrus here accepts only ONE sync-wait per instruction: extra waits are
    peeled onto same-engine NoOps (_split_multiwait_insts)

Self-contained: hardcodes shapes from the problem spec.
"""

import sys

for _p in ("/opt/trn_rl_repo",):
    if _p not in sys.path:
        sys.path.insert(0, _p)

from contextlib import ExitStack

import numpy as np

import concourse.bass as bass
import concourse.mybir as mybir
import concourse.tile as tile
import bass_rust


FP16 = mybir.dt.float16
FP32 = mybir.dt.float32

P = 128
NCORES = 8
B, S, IN, OUT = 4, 2048, 4096, 4096
GROUP = 128
M = B * S // NCORES          # 1024 rows of x per core
KO = IN // P                 # 32 k-chunks == quant groups
OC = 512                     # o-chunk (matmul free dim)
NOC = OUT // OC              # 8
MT = M // P                  # 8 m-tiles
SPAN = 2                     # panels staged per store

_RUNNER = None


def _split_multiwait_insts(nc):
    """This env's walrus CoreV3 codegen accepts only one sync-wait per
    instruction; Tile's tail drain can carry one per DMAHW sem lane.
    Peel extra waits onto same-engine NoOps inserted just before."""
    ctr = 0
    for f in nc.m.functions:
        for bb in f.blocks:
            new = []
            for inst in bb.instructions:
                si = inst.sync_info
                if si is not None and si.on_wait and len(si.on_wait) > 1:
                    waits = list(si.on_wait)
                    for w in waits[:-1]:
                        ctr += 1
                        new.append(bass_rust.InstNoOp(
                            name=f"I-waitsplit-{ctr}",
                            engine=inst.engine,
                            sync_info=bass_rust.SyncInfo(on_wait=[w], on_update=[]),
                        ))
                    inst.sync_info = bass_rust.SyncInfo(
                        on_wait=[waits[-1]], on_update=list(si.on_update or [])
                    )
                new.append(inst)
            bb.instructions = new
    return ctr


def _build(scb_bufs=1, wdeq_bufs=2, psum_bufs=8, xchunks=4, mul_split=2):
    nc = bass.Bass()
    x = nc.declare_dram_parameter("x", [M, IN], FP16, isOutput=False)
    w = nc.declare_dram_parameter("w", [OUT, IN], FP16, isOutput=False)
    sb = nc.declare_dram_parameter("sb", [P, NOC * KO * OC], FP16, isOutput=False)
    y = nc.declare_dram_parameter("y", [M, OUT], FP16, isOutput=True)

    with tile.TileContext(nc) as tc, ExitStack() as ctx:
        xTp = ctx.enter_context(tc.tile_pool(name="xTp", bufs=1))
        wraw = ctx.enter_context(tc.tile_pool(name="wraw", bufs=wdeq_bufs))
        scbp = ctx.enter_context(tc.tile_pool(name="scbp", bufs=scb_bufs))
        ystg = ctx.enter_context(tc.tile_pool(name="ystg", bufs=2))
        psum = ctx.enter_context(tc.tile_pool(name="psum", bufs=psum_bufs,
                                              space="PSUM"))

        xT = xTp.tile([P, KO, M], FP16)

        def emit_xchunk(i):
            xc = KO // xchunks
            nc.sync.dma_start_transpose(out=xT[:, i * xc:(i + 1) * xc, :],
                                        in_=x[:, i * xc * P:(i + 1) * xc * P])

        def emit_prefetch(oc):
            """Transpose w panel, load its broadcast scales, dequantize."""
            osl = slice(oc * OC, (oc + 1) * OC)
            wr3 = wraw.tile([P, KO, OC], FP16, tag="wraw", name="wr3")
            scb = scbp.tile([P, KO, OC], FP16, tag="scb", name="scb")
            # Panel 0's scales go via SWDGE (gpsimd); later panels ride the
            # scalar HWDGE ring. Tile serializes plain copies against
            # in-flight DMA-transposes (xbar-deadlock guard), so the scales
            # loads and w transposes alternate on the wire; inside the 55 us
            # panel window that's harmless.
            eng = nc.gpsimd if oc == 0 else nc.scalar
            eng.dma_start(out=scb[:],
                          in_=sb[:, oc * KO * OC:(oc + 1) * KO * OC])
            KH = KO // mul_split
            for g in range(mul_split):
                kg = slice(g * KH, (g + 1) * KH)
                nc.sync.dma_start_transpose(
                    out=wr3[:, kg, :], in_=w[osl, g * KH * P:(g + 1) * KH * P])
                nc.vector.tensor_mul(wr3[:, kg, :], wr3[:, kg, :], scb[:, kg, :])
            return wr3

        stage = {}

        def emit_compute(oc, wr3):
            grp, ph = oc // SPAN, oc % SPAN
            if ph == 0:
                stage[grp] = ystg.tile([P, MT, SPAN, OC], FP16, name="yst")
            yst = stage[grp]
            if oc == 0:
                # Panel 0 runs ko-pass-major across all 8 psum banks so the
                # matmul stream starts as soon as x chunk 0 + w half 0 land,
                # instead of the first m-chain serializing on every x chunk.
                pts = [psum.tile([P, OC], FP32, name="pt") for _ in range(MT)]
                npass = 4
                kp = KO // npass
                for p_i in range(npass):
                    for m in range(MT):
                        for k2 in range(p_i * kp, (p_i + 1) * kp):
                            nc.tensor.matmul(
                                pts[m][:],
                                xT[:, k2, m * P:(m + 1) * P],
                                wr3[:, k2, :],
                                start=(k2 == 0),
                                stop=(k2 == KO - 1),
                            )
                for m in range(MT):
                    nc.scalar.copy(out=yst[:, m, ph, :], in_=pts[m][:])
            else:
                for m in range(MT):
                    pt = psum.tile([P, OC], FP32, name="pt")
                    for ko in range(KO):
                        nc.tensor.matmul(
                            pt[:],
                            xT[:, ko, m * P:(m + 1) * P],
                            wr3[:, ko, :],
                            start=(ko == 0),
                            stop=(ko == KO - 1),
                        )
                    nc.scalar.copy(out=yst[:, m, ph, :], in_=pt[:])
            if ph == SPAN - 1:
                for m in range(MT):
                    nc.scalar.dma_start(
                        out=y[m * P:(m + 1) * P,
                              grp * SPAN * OC:(grp + 1) * SPAN * OC],
                        in_=yst[:, m, :, :])

        emit_xchunk(0)
        cur = emit_prefetch(0)
        for i in range(1, xchunks):
            emit_xchunk(i)
        for oc in range(NOC):
            nxt = emit_prefetch(oc + 1) if oc + 1 < NOC else None
            emit_compute(oc, cur)
            cur = nxt

    _split_multiwait_insts(nc)
    return nc


def make_sb(scales):
    """[OUT, KO] fp16 scales -> [128, NOC*KO*OC] panel-major broadcast."""
    pan = np.ascontiguousarray(
        np.asarray(scales, dtype=np.float16).T.reshape(KO, NOC, OC)
        .transpose(1, 0, 2))
    return np.ascontiguousarray(
        np.broadcast_to(pan.reshape(1, -1), (P, NOC * KO * OC)))


def make_in_maps(x, weight, scales):
    """Full inputs -> per-core input maps for the bass kernel."""
    xf = np.ascontiguousarray(np.asarray(x, dtype=np.float16).reshape(B * S, IN))
    w = np.ascontiguousarray(np.asarray(weight, dtype=np.float16))
    sbv = make_sb(scales)
    return [
        {"x": xf[c * M:(c + 1) * M], "w": w, "sb": sbv} for c in range(NCORES)
    ]


def _get_runner():
    """Compile once; return a reusable callable mapping per-core input maps
    to per-core output maps (modeled on bass2jax.run_bass_via_pjrt)."""
    global _RUNNER
    if _RUNNER is not None:
        return _RUNNER

    import jax
    from jax.experimental.shard_map import shard_map
    from jax.sharding import Mesh, PartitionSpec
    from concourse import bass2jax

    nc = _build()
    bass2jax.install_neuronx_cc_hook()

    partition_name = nc.partition_id_tensor.name if nc.partition_id_tensor else None
    in_names, out_names, out_avals, zero_shapes = [], [], [], []
    for alloc in nc.m.functions[0].allocations:
        if not isinstance(alloc, mybir.MemoryLocationSet):
            continue
        name = alloc.memorylocations[0].name
        if alloc.kind == "ExternalInput":
            if name != partition_name:
                in_names.append(name)
        elif alloc.kind == "ExternalOutput":
            shape = tuple(alloc.tensor_shape)
            dtype = mybir.dt.np(alloc.dtype)
            out_names.append(name)
            out_avals.append(jax.core.ShapedArray(shape, dtype))
            zero_shapes.append((shape, dtype))
    n_params = len(in_names)
    n_outs = len(out_names)
    all_names = in_names + out_names
    if partition_name is not None:
        all_names = all_names + [partition_name]
    donate = tuple(range(n_params, n_params + n_outs))

    def _make_body(reps):
        def _body(*args):
            ins = list(args[:n_params])
            outs = list(args[n_params:n_params + n_outs])
            for _ in range(reps):
                operands = ins + outs
                if partition_name is not None:
                    operands.append(bass2jax.partition_id_tensor())
                outs = list(bass2jax._bass_exec_p.bind(
                    *operands,
                    out_avals=tuple(out_avals),
                    in_names=tuple(all_names),
                    out_names=tuple(out_names),
                    lowering_input_output_aliases=(),
                    sim_require_finite=True,
                    sim_require_nnan=True,
                    nc=nc,
                ))
            return tuple(outs)
        return _body

    devices = jax.devices()[:NCORES]
    mesh = Mesh(np.asarray(devices), ("core",))

    def _make_exec(reps):
        return jax.jit(
            shard_map(
                _make_body(reps),
                mesh=mesh,
                in_specs=(PartitionSpec("core"),) * (n_params + n_outs),
                out_specs=(PartitionSpec("core"),) * n_outs,
                check_rep=False,
            ),
            donate_argnums=donate,
            keep_unused=True,
        )

    sharded = _make_exec(1)
    _exec_cache = {1: sharded}
    from jax.sharding import NamedSharding
    shard = NamedSharding(mesh, PartitionSpec("core"))

    class Runner:
        def __init__(self):
            self.in_names = in_names
            self.out_names = out_names

        def put_inputs(self, in_maps):
            """Concat per-core inputs and place them on the mesh."""
            import jax as _jax
            concat_in = [
                np.concatenate([np.asarray(m[name]) for m in in_maps], axis=0)
                for name in in_names
            ]
            return [_jax.device_put(a, shard) for a in concat_in]

        def fresh_outs(self):
            import jax as _jax
            return [
                _jax.device_put(np.zeros((NCORES * sh[0], *sh[1:]), dt), shard)
                for sh, dt in zero_shapes
            ]

        def exec_dev(self, dev_in, dev_outs, reps=1):
            """Device step(s). dev_outs is donated; returns new out arrays
            (same shape/sharding — reusable as the next call's dev_outs,
            since the kernel overwrites every output element). reps>1
            chains that many NEFF executions inside one dispatch."""
            if reps not in _exec_cache:
                _exec_cache[reps] = _make_exec(reps)
            return _exec_cache[reps](*dev_in, *dev_outs)

        def run(self, in_maps):
            dev_in = self.put_inputs(in_maps)
            out_arrs = self.exec_dev(dev_in, self.fresh_outs())
            return [
                {
                    name: np.asarray(out_arrs[i]).reshape(
                        NCORES, *out_avals[i].shape)[c]
                    for i, name in enumerate(out_names)
                }
                for c in range(NCORES)
            ]

    _RUNNER = Runner()
    return _RUNNER


def kernel(x, weight, scales):
    runner = _get_runner()
    in_maps = make_in_maps(x, weight, scales)
    outs = runner.run(in_maps)
    yf = np.concatenate([outs[c]["y"] for c in range(NCORES)], axis=0)
    return yf.reshape(B, S, OUT).astype(np.float16)


# revision 14
# speedup vs baseline: 1.9016x; 1.0151x over previous
"""Grouped-scale dequant GEMM (AxCoreLinearFP16) on 8 Trainium2 NeuronCores.

y[b,s,o] = sum_i x[b,s,i] * (weight[o,i] * scales[o, i//128])

Strategy: data-parallel over the flattened (b*s) rows — each core gets a
[1024, 4096] x-shard and the full weight/scales (no collectives). Per core:
  - x^T resident in SBUF via four 2 MiB DMA transposes (contraction dim on
    partitions: xT[p, ko, m] = x[m, ko*128+p]); 4 chunks so the first
    matmuls can start after ~8 µs instead of ~35.
  - w^T o-panels (512 wide) DMA-transposed in two 2 MiB chunks each.
  - dequant: the host pre-transposes scales to panel-major [KO, OC] slices
    and replicates them across 128 partitions ("sb" input, 32 MiB). Per
    panel ONE plain 4 MiB DMA loads the broadcast scales, and two all-fp16
    DVE tensor_muls (2x-rate: 16-bit, step-1 operands) dequantize the w^T
    panel in place. This keeps the PE stream free of the rank-1 broadcast
    matmuls the previous version used, and keeps DVE at 2x (the old
    fp32-PSUM operand forced 1x and made dequant the critical path).
  - PE matmul accumulates over the 32 k-chunks into PSUM [128, 512] fp32;
    8-bank psum pool; PSUM evicted with casting ACT copies into a 2-panel
    staging buffer, stored to DRAM as [128, 2, 512] tiles (2 KiB contiguous
    per output row). Direct per-panel 1 KiB-row stores measurably push the
    chip into a power state where the PE drops 2.4 -> 2.0 GHz (matmuls
    455 -> 259... sorry, 216 -> 259 ns); the staged layout avoids that.

Workarounds for this environment's toolchain:
  - walrus here accepts only ONE sync-wait per instruction: extra waits are
    peeled onto same-engine NoOps (_split_multiwait_insts)

Self-contained: hardcodes shapes from the problem spec.
"""

import sys

for _p in ("/opt/trn_rl_repo",):
    if _p not in sys.path:
        sys.path.insert(0, _p)

from contextlib import ExitStack

import numpy as np

import concourse.bass as bass
import concourse.mybir as mybir
import concourse.tile as tile
import bass_rust


FP16 = mybir.dt.float16
FP32 = mybir.dt.float32

P = 128
NCORES = 8
B, S, IN, OUT = 4, 2048, 4096, 4096
GROUP = 128
M = B * S // NCORES          # 1024 rows of x per core
KO = IN // P                 # 32 k-chunks == quant groups
OC = 512                     # o-chunk (matmul free dim)
NOC = OUT // OC              # 8
MT = M // P                  # 8 m-tiles
SPAN = 2                     # panels staged per store

_RUNNER = None


def _split_multiwait_insts(nc):
    """This env's walrus CoreV3 codegen accepts only one sync-wait per
    instruction; Tile's tail drain can carry one per DMAHW sem lane.
    Peel extra waits onto same-engine NoOps inserted just before."""
    ctr = 0
    for f in nc.m.functions:
        for bb in f.blocks:
            new = []
            for inst in bb.instructions:
                si = inst.sync_info
                if si is not None and si.on_wait and len(si.on_wait) > 1:
                    waits = list(si.on_wait)
                    for w in waits[:-1]:
                        ctr += 1
                        new.append(bass_rust.InstNoOp(
                            name=f"I-waitsplit-{ctr}",
                            engine=inst.engine,
                            sync_info=bass_rust.SyncInfo(on_wait=[w], on_update=[]),
                        ))
                    inst.sync_info = bass_rust.SyncInfo(
                        on_wait=[waits[-1]], on_update=list(si.on_update or [])
                    )
                new.append(inst)
            bb.instructions = new
    return ctr


def _build(scb_bufs=1, wdeq_bufs=2, psum_bufs=8, xchunks=4, mul_split=2):
    nc = bass.Bass()
    x = nc.declare_dram_parameter("x", [M, IN], FP16, isOutput=False)
    w = nc.declare_dram_parameter("w", [OUT, IN], FP16, isOutput=False)
    sb = nc.declare_dram_parameter("sb", [P, NOC * KO * OC], FP16, isOutput=False)
    y = nc.declare_dram_parameter("y", [M, OUT], FP16, isOutput=True)

    with tile.TileContext(nc) as tc, ExitStack() as ctx:
        xTp = ctx.enter_context(tc.tile_pool(name="xTp", bufs=1))
        wraw = ctx.enter_context(tc.tile_pool(name="wraw", bufs=wdeq_bufs))
        scbp = ctx.enter_context(tc.tile_pool(name="scbp", bufs=scb_bufs))
        ystg = ctx.enter_context(tc.tile_pool(name="ystg", bufs=2))
        psum = ctx.enter_context(tc.tile_pool(name="psum", bufs=psum_bufs,
                                              space="PSUM"))

        xT = xTp.tile([P, KO, M], FP16)

        def emit_xchunk(i):
            xc = KO // xchunks
            nc.sync.dma_start_transpose(out=xT[:, i * xc:(i + 1) * xc, :],
                                        in_=x[:, i * xc * P:(i + 1) * xc * P])

        def emit_prefetch(oc):
            """Transpose w panel, load its broadcast scales, dequantize."""
            osl = slice(oc * OC, (oc + 1) * OC)
            wr3 = wraw.tile([P, KO, OC], FP16, tag="wraw", name="wr3")
            scb = scbp.tile([P, KO, OC], FP16, tag="scb", name="scb")
            # Panel 0's scales go via SWDGE (gpsimd); later panels ride the
            # scalar HWDGE ring. Tile serializes plain copies against
            # in-flight DMA-transposes (xbar-deadlock guard), so the scales
            # loads and w transposes alternate on the wire; inside the 55 us
            # panel window that's harmless.
            # Panel 0's scales go via SWDGE (gpsimd); later panels ride the
            # scalar HWDGE ring. Tile serializes plain copies against
            # in-flight DMA-transposes (xbar-deadlock guard), so the scales
            # loads and w transposes alternate on the wire; inside the 55 us
            # panel window that's harmless.
            eng = nc.gpsimd if oc == 0 else nc.scalar
            eng.dma_start(out=scb[:],
                          in_=sb[:, oc * KO * OC:(oc + 1) * KO * OC])
            KH = KO // mul_split
            for g in range(mul_split):
                kg = slice(g * KH, (g + 1) * KH)
                # NB: transposes must stay on nc.sync — dma_start_transpose
                # issued from the scalar/ACT ring returns garbage here.
                nc.sync.dma_start_transpose(
                    out=wr3[:, kg, :], in_=w[osl, g * KH * P:(g + 1) * KH * P])
                nc.vector.tensor_mul(wr3[:, kg, :], wr3[:, kg, :], scb[:, kg, :])
            return wr3

        stage = {}

        def emit_compute(oc, wr3):
            grp, ph = oc // SPAN, oc % SPAN
            if ph == 0:
                stage[grp] = ystg.tile([P, MT, SPAN, OC], FP16, name="yst")
            yst = stage[grp]
            if oc == 0:
                # Panel 0 runs ko-pass-major across all 8 psum banks so the
                # matmul stream starts as soon as x chunk 0 + w half 0 land,
                # instead of the first m-chain serializing on every x chunk.
                pts = [psum.tile([P, OC], FP32, name="pt") for _ in range(MT)]
                npass = 4
                kp = KO // npass
                for p_i in range(npass):
                    for m in range(MT):
                        for k2 in range(p_i * kp, (p_i + 1) * kp):
                            nc.tensor.matmul(
                                pts[m][:],
                                xT[:, k2, m * P:(m + 1) * P],
                                wr3[:, k2, :],
                                start=(k2 == 0),
                                stop=(k2 == KO - 1),
                            )
                for m in range(MT):
                    nc.scalar.copy(out=yst[:, m, ph, :], in_=pts[m][:])
            else:
                for m in range(MT):
                    pt = psum.tile([P, OC], FP32, name="pt")
                    for ko in range(KO):
                        nc.tensor.matmul(
                            pt[:],
                            xT[:, ko, m * P:(m + 1) * P],
                            wr3[:, ko, :],
                            start=(ko == 0),
                            stop=(ko == KO - 1),
                        )
                    nc.scalar.copy(out=yst[:, m, ph, :], in_=pt[:])
            if ph == SPAN - 1:
                for m in range(MT):
                    nc.scalar.dma_start(
                        out=y[m * P:(m + 1) * P,
                              grp * SPAN * OC:(grp + 1) * SPAN * OC],
                        in_=yst[:, m, :, :])

        emit_xchunk(0)
        cur = emit_prefetch(0)
        for i in range(1, xchunks):
            emit_xchunk(i)
        for oc in range(NOC):
            nxt = emit_prefetch(oc + 1) if oc + 1 < NOC else None
            emit_compute(oc, cur)
            cur = nxt

    _split_multiwait_insts(nc)
    return nc


def make_sb(scales):
    """[OUT, KO] fp16 scales -> [128, NOC*KO*OC] panel-major broadcast."""
    pan = np.ascontiguousarray(
        np.asarray(scales, dtype=np.float16).T.reshape(KO, NOC, OC)
        .transpose(1, 0, 2))
    return np.ascontiguousarray(
        np.broadcast_to(pan.reshape(1, -1), (P, NOC * KO * OC)))


def make_in_maps(x, weight, scales):
    """Full inputs -> per-core input maps for the bass kernel."""
    xf = np.ascontiguousarray(np.asarray(x, dtype=np.float16).reshape(B * S, IN))
    w = np.ascontiguousarray(np.asarray(weight, dtype=np.float16))
    sbv = make_sb(scales)
    return [
        {"x": xf[c * M:(c + 1) * M], "w": w, "sb": sbv} for c in range(NCORES)
    ]


def _get_runner():
    """Compile once; return a reusable callable mapping per-core input maps
    to per-core output maps (modeled on bass2jax.run_bass_via_pjrt)."""
    global _RUNNER
    if _RUNNER is not None:
        return _RUNNER

    import jax
    from jax.experimental.shard_map import shard_map
    from jax.sharding import Mesh, PartitionSpec
    from concourse import bass2jax

    nc = _build()
    bass2jax.install_neuronx_cc_hook()

    partition_name = nc.partition_id_tensor.name if nc.partition_id_tensor else None
    in_names, out_names, out_avals, zero_shapes = [], [], [], []
    for alloc in nc.m.functions[0].allocations:
        if not isinstance(alloc, mybir.MemoryLocationSet):
            continue
        name = alloc.memorylocations[0].name
        if alloc.kind == "ExternalInput":
            if name != partition_name:
                in_names.append(name)
        elif alloc.kind == "ExternalOutput":
            shape = tuple(alloc.tensor_shape)
            dtype = mybir.dt.np(alloc.dtype)
            out_names.append(name)
            out_avals.append(jax.core.ShapedArray(shape, dtype))
            zero_shapes.append((shape, dtype))
    n_params = len(in_names)
    n_outs = len(out_names)
    all_names = in_names + out_names
    if partition_name is not None:
        all_names = all_names + [partition_name]
    donate = tuple(range(n_params, n_params + n_outs))

    def _make_body(reps):
        def _body(*args):
            ins = list(args[:n_params])
            outs = list(args[n_params:n_params + n_outs])
            for _ in range(reps):
                operands = ins + outs
                if partition_name is not None:
                    operands.append(bass2jax.partition_id_tensor())
                outs = list(bass2jax._bass_exec_p.bind(
                    *operands,
                    out_avals=tuple(out_avals),
                    in_names=tuple(all_names),
                    out_names=tuple(out_names),
                    lowering_input_output_aliases=(),
                    sim_require_finite=True,
                    sim_require_nnan=True,
                    nc=nc,
                ))
            return tuple(outs)
        return _body

    devices = jax.devices()[:NCORES]
    mesh = Mesh(np.asarray(devices), ("core",))

    def _make_exec(reps):
        return jax.jit(
            shard_map(
                _make_body(reps),
                mesh=mesh,
                in_specs=(PartitionSpec("core"),) * (n_params + n_outs),
                out_specs=(PartitionSpec("core"),) * n_outs,
                check_rep=False,
            ),
            donate_argnums=donate,
            keep_unused=True,
        )

    sharded = _make_exec(1)
    _exec_cache = {1: sharded}
    from jax.sharding import NamedSharding
    shard = NamedSharding(mesh, PartitionSpec("core"))

    class Runner:
        def __init__(self):
            self.in_names = in_names
            self.out_names = out_names

        def put_inputs(self, in_maps):
            """Concat per-core inputs and place them on the mesh."""
            import jax as _jax
            concat_in = [
                np.concatenate([np.asarray(m[name]) for m in in_maps], axis=0)
                for name in in_names
            ]
            return [_jax.device_put(a, shard) for a in concat_in]

        def fresh_outs(self):
            import jax as _jax
            return [
                _jax.device_put(np.zeros((NCORES * sh[0], *sh[1:]), dt), shard)
                for sh, dt in zero_shapes
            ]

        def exec_dev(self, dev_in, dev_outs, reps=1):
            """Device step(s). dev_outs is donated; returns new out arrays
            (same shape/sharding — reusable as the next call's dev_outs,
            since the kernel overwrites every output element). reps>1
            chains that many NEFF executions inside one dispatch."""
            if reps not in _exec_cache:
                _exec_cache[reps] = _make_exec(reps)
            return _exec_cache[reps](*dev_in, *dev_outs)

        def run(self, in_maps):
            dev_in = self.put_inputs(in_maps)
            out_arrs = self.exec_dev(dev_in, self.fresh_outs())
            return [
                {
                    name: np.asarray(out_arrs[i]).reshape(
                        NCORES, *out_avals[i].shape)[c]
                    for i, name in enumerate(out_names)
                }
                for c in range(NCORES)
            ]

    _RUNNER = Runner()
    return _RUNNER


def kernel(x, weight, scales):
    runner = _get_runner()
    in_maps = make_in_maps(x, weight, scales)
    outs = runner.run(in_maps)
    yf = np.concatenate([outs[c]["y"] for c in range(NCORES)], axis=0)
    return yf.reshape(B, S, OUT).astype(np.float16)
